# revision 51
# baseline (speedup 1.0000x reference)
"""AttentionPairBias Trainium2 Bass kernel.

Problem: nn_AttentionPairBias_49486613184627
  B=2, N=1024, D=768, E=128, H=16, HD=48.

Sharding: query-row (i) sharding across 8 cores. Core c handles rows
i in [c*128, (c+1)*128) for both batches. Each core reads its edge_embed
shard (67MB in fp16), full k_in (dup k/v projection), and produces its
(2,128,768) slice of the output.

Device-side layout strategy ("etp", interleaved emission):
  - edge is pre-transposed on host to (B, IS, E, N): tiles load as
    [e(part), j(free)] directly -> no PE transposes, no psum->sbuf
    copies for the bias path.
  - pair bias per (b,i,jc): one matmul lhsT=edge_jc rhs=[wza' | ones]
    where wza' = ln_g*Wz column-demeaned on host -- this folds the
    LayerNorm mean subtraction into the weights (sum_e (x-mu)w =
    sum_e x(w-mean(w))). P[j,0:16]=mean-centered bias, P[j,16]=sum_e x
    (for the variance only). The edge tile is squared (split across
    DVE/ACT/GpSimd); a matmul lhsT=esq_jc rhs=ones accumulates
    sum_e x^2 into a per-16-i-block stats psum tile (contiguous). The
    LayerNorm scale is a single batched fixup bias = P*rstd (GpSimd).
  - scores tiles are [j(part), i(free)] per (b,h,jc); bias added on DVE,
    exp on ACT -> fp16 sbuf.
  - o = exp^T @ [v | ones]: ones column fused into v (49-wide heads) so
    col 48 of the o-psum is the softmax denominator; 1/s is a
    per-partition ACT scale.
  - emission is interleaved so engines overlap across phases:
    [proj-phase || bias-sweep(b0)], then [attn(b0) || bias-sweep(b1)],
    then attn(b1); abuf is double-buffered to decouple the batches.
  - ACT uses only Identity/Ln/Exp/Square -> one table set, no reloads.
    (sigmoid computed as 1/(1+exp(-z)) with DVE reciprocal)
"""

import os
import sys

import numpy as np

for _p in ("/opt/trn_rl_repo",):
    if _p not in sys.path:
        sys.path.insert(0, _p)

import concourse.bacc as bacc
import concourse.bass as bass
import concourse.mybir as mybir
import concourse.tile as tile
from concourse.bass_utils import run_bass_kernel_spmd

F16 = mybir.dt.float16
F32 = mybir.dt.float32
AF = mybir.ActivationFunctionType
ALU = mybir.AluOpType

B, N, D, E, H = 2, 1024, 768, 128, 16
HD = 48
HDP = 64              # padded head dim (projection layout)
VW = HD + 1           # v width per head incl. ones column
DP = H * HDP          # 1024 padded model dim
NC = 8                # cores
IS = N // NC          # 128 i-rows per core per batch
JC = N // 128         # 8 j-chunks
MC = D // 128         # 6 contraction chunks of 128 over D
IBLK = 16             # i-batch for stats/fixup
SW = 17               # bias matmul width: 16 bias cols + sum_e x
EPS = 1e-5

_BUILT = None         # cached program
LAST_RESULTS = None   # BassKernelResults of last run (for test.py)


def _build_program():
    nc = bacc.Bacc(
        "TRN2",
        target_bir_lowering=False,
        debug=False,
        enable_asserts=False,
        num_devices=NC,
    )

    # ---------------- DRAM I/O ----------------
    d_edge = nc.dram_tensor("e", (B, IS, E, N), F16, kind="ExternalInput").ap()
    d_xt = nc.dram_tensor("xt", (B, D, IS), F16, kind="ExternalInput").ap()
    d_kin = nc.dram_tensor("kin", (B, D, N), F16, kind="ExternalInput").ap()
    d_wq = nc.dram_tensor("wq", (D, DP), F16, kind="ExternalInput").ap()
    d_wk = nc.dram_tensor("wk", (D, DP), F16, kind="ExternalInput").ap()
    d_wv = nc.dram_tensor("wv", (D, DP), F16, kind="ExternalInput").ap()
    d_wg = nc.dram_tensor("wg", (D, DP), F16, kind="ExternalInput").ap()
    d_wo = nc.dram_tensor("wo", (DP, D), F16, kind="ExternalInput").ap()
    d_bq = nc.dram_tensor("bq", (HDP * H // 128, 128), F32, kind="ExternalInput").ap()
    d_wza = nc.dram_tensor("wza", (E, SW), F16, kind="ExternalInput").ap()
    d_id16 = nc.dram_tensor("id16", (128, 128), F16, kind="ExternalInput").ap()
    d_out = nc.dram_tensor("o", (B, IS, D), F16, kind="ExternalOutput").ap()

    from contextlib import ExitStack

    with tile.TileContext(nc) as tc, ExitStack() as es:
        def pool(**kw):
            return es.enter_context(tc.tile_pool(**kw))

        # ---- SBUF pools ----
        constp = pool(name="const", bufs=1)
        ktpp = pool(name="ktp", bufs=1)
        vallp = pool(name="vall", bufs=1)
        qtpp = pool(name="qtp", bufs=1)
        gallp = pool(name="gall", bufs=1)
        wosbp = pool(name="wo_sb", bufs=1)
        wchp = pool(name="wchunk", bufs=6)
        kinchp = pool(name="kinchunk", bufs=6)
        gwork = pool(name="gwork", bufs=1)
        abufp = pool(name="abuf", bufs=2)
        etpp = pool(name="etp", bufs=3)
        pbufp = pool(name="pbuf", bufs=3)
        smallp = pool(name="small", bufs=2)
        lnvp = pool(name="lnv", bufs=3)
        expsbp = pool(name="expsb", bufs=2)
        oasmp = pool(name="oasm", bufs=1)
        goTp = pool(name="goT", bufs=1)
        outsbp = pool(name="outsb", bufs=1)
        # ---- PSUM pools (8 banks) ----
        ppps = pool(name="pp_ps", bufs=2, space="PSUM")   # P: f32 136
        sqps = pool(name="sq_ps", bufs=2, space="PSUM")   # sumsq f32 128
        mmps = pool(name="mm_ps", bufs=2, space="PSUM")   # proj/scores f32 512
        ops = pool(name="o_ps", bufs=2, space="PSUM")     # o+s: f32 49

        # ============ constants ============
        id16 = constp.tile([128, 128], F16)
        nc.sync.dma_start(id16[:], d_id16[:, :])
        wza = constp.tile([E, SW], F16)
        nc.sync.dma_start(wza[:], d_wza[:, :])
        bqp = constp.tile([128, DP // 128], F32)
        nc.sync.dma_start(bqp[:], d_bq.rearrange("m p -> p m"))
        ones16 = constp.tile([128, 1], F16)
        nc.vector.memset(ones16[:], 1.0)
        epsc = constp.tile([128, 1], F32)
        nc.vector.memset(epsc[:], EPS)

        # persistent activation buffers
        # ktp: [b][m] 128 x 1024 (d' rows, j cols), fp16
        ktp = ktpp.tile([128, B * 8 * 1024], F16)
        ktp3 = ktp[:].rearrange("p (b m j) -> p b m j", b=B, m=8)
        # v: [b][jt][h] 128 x 49 (j rows, [v|ones] cols), fp16
        vall = vallp.tile([128, B * 8 * H * VW], F16)
        vall4 = vall[:].rearrange(
            "p (b jt h c) -> p b jt h c", b=B, jt=8, h=H
        )
        nc.vector.memset(vall4[:, :, :, :, HD:HD + 1], 1.0)
        # qtp: [m] 128 x (b,i), fp16
        qtp = qtpp.tile([128, 8 * B * IS], F16)
        qtp3 = qtp[:].rearrange("p (m b i) -> p m b i", m=8, b=B)
        # g: [b] 128(i) x 1024(d'), fp16
        gall = gallp.tile([128, B * DP], F16)
        gall2 = gall[:].rearrange("p (b d) -> p b d", b=B)
        # wo chunks: [cc] 128 x 768 fp16
        wosb = wosbp.tile([128, 8 * D], F16)
        wosb2 = wosb[:].rearrange("p (c d) -> p c d", c=8)
        nc.sync.dma_start(wosb2, d_wo.rearrange("(c p) d -> p c d", p=128))
        # xt tiles: [c] 128(d-row) x (b,i)
        xts = constp.tile([128, MC * B * IS], F16)
        xts3 = xts[:].rearrange("p (c b i) -> p c b i", c=MC, b=B)
        for b in range(B):
            for c in range(MC):
                nc.sync.dma_start(
                    xts3[:, c, b, :], d_xt[b, c * 128:(c + 1) * 128, :]
                )

        # ============ phase 0 generator: projections ============
        def load_chunks(dram, tag, n=MC, width=DP):
            ts = []
            for c in range(n):
                t = wchp.tile([128, width], F16, tag=tag)
                nc.sync.dma_start(t[:], dram[c * 128:(c + 1) * 128, :])
                ts.append(t)
            return ts

        def ph0_gen():
            # q projection (both b at once; xts free dim is (b,i))
            wq_sb = load_chunks(d_wq, "w")
            for m in range(8):
                qps = mmps.tile([128, B * IS], F32, tag="sc")
                for c in range(MC):
                    nc.tensor.matmul(
                        qps[:],
                        wq_sb[c][:, m * 128:(m + 1) * 128],
                        xts3[:, c, :, :],
                        start=(c == 0), stop=(c == MC - 1),
                    )
                nc.scalar.activation(
                    qtp3[:, m, :, :], qps[:],
                    AF.Identity, bias=bqp[:, m:m + 1], scale=1.0,
                )
                yield
            # g = 1/(1+exp(-z)); wg is pre-negated on host -> psum = -z
            wg_sb = load_chunks(d_wg, "w")
            for b in range(B):
                for nb in range(2):
                    gps = mmps.tile([128, 512], F32, tag="sc")
                    for c in range(MC):
                        nc.tensor.matmul(
                            gps[:],
                            xts3[:, c, b, :],
                            wg_sb[c][:, nb * 512:(nb + 1) * 512],
                            start=(c == 0), stop=(c == MC - 1),
                        )
                    gtmp = gwork.tile([128, 512], F32, tag="gtmp")
                    nc.scalar.activation(
                        gtmp[:], gps[:], AF.Exp, bias=0.0, scale=1.0,
                    )
                    nc.vector.tensor_scalar_add(gtmp[:], gtmp[:], 1.0)
                    with nc.allow_low_precision(
                        reason="sigmoid gate in (0,1); fp16 out is ample"
                    ):
                        nc.vector.reciprocal(
                            gall2[:, b, nb * 512:(nb + 1) * 512], gtmp[:]
                        )
                    yield
            # per-b: k^T and v
            for b in range(B):
                kin_sb = []
                for c in range(MC):
                    t = kinchp.tile([128, N], F16, tag="kin")
                    nc.sync.dma_start(
                        t[:], d_kin[b, c * 128:(c + 1) * 128, :]
                    )
                    kin_sb.append(t)
                wk_sb = load_chunks(d_wk, "w")
                for m in range(8):
                    for nb in range(2):
                        kps = mmps.tile([128, 512], F32, tag="sc")
                        for c in range(MC):
                            nc.tensor.matmul(
                                kps[:],
                                wk_sb[c][:, m * 128:(m + 1) * 128],
                                kin_sb[c][:, nb * 512:(nb + 1) * 512],
                                start=(c == 0), stop=(c == MC - 1),
                            )
                        nc.scalar.activation(
                            ktp3[:, b, m, nb * 512:(nb + 1) * 512], kps[:],
                            AF.Identity, bias=0.0, scale=1.0,
                        )
                        yield
                wv_sb = load_chunks(d_wv, "w")
                for jt in range(8):
                    for nb in range(2):
                        vps = mmps.tile([128, 512], F32, tag="sc")
                        for c in range(MC):
                            nc.tensor.matmul(
                                vps[:],
                                kin_sb[c][:, jt * 128:(jt + 1) * 128],
                                wv_sb[c][:, nb * 512:(nb + 1) * 512],
                                start=(c == 0), stop=(c == MC - 1),
                            )
                        nc.scalar.activation(
                            vall4[:, b, jt, nb * 8:(nb + 1) * 8, 0:HD],
                            vps[:].rearrange(
                                "p (h c) -> p h c", h=8
                            )[:, :, 0:HD],
                            AF.Identity, bias=0.0, scale=1.0,
                        )
                        yield

        # ============ bias sweep generator ============
        NI = 2                      # i-rows per sweep step
        def sweep_gen(b, abuf3, gp_steps=(0, 5, 10), act_steps=(3, 8, 13)):
            # deferred emission: sq-matmuls one step late; rstd Exp +
            # fixup batched per 2 iblks (batches the Ln ops apart from
            # the Exp ops -> far fewer activation-table reloads, and no
            # engine queue head-of-line-blocks on a fresh dependency)
            pending_sqmm = []   # list of per-step lists; flushed lag-2
            pending_fixup = []

            def flush_sqmm(keep=0):
                while len(pending_sqmm) > keep:
                    for out_ap, lhsT in pending_sqmm.pop(0):
                        nc.tensor.matmul(out_ap, lhsT, ones16[:],
                                         start=True, stop=True)

            def flush_fixup():
                for lnv, ab_blk, p_r in pending_fixup:
                    rstd = lnvp.tile([128, IBLK * JC], F16, tag="rs")
                    nc.scalar.activation(
                        rstd[:], lnv[:], AF.Exp, bias=0.0, scale=-0.5
                    )
                    rstd3 = rstd[:].rearrange(
                        "p (i jc) -> p i jc", i=IBLK
                    )
                    r_bc = rstd3.rearrange(
                        "p i jc -> p jc () i"
                    ).broadcast_to((128, JC, 16, IBLK))
                    nc.vector.tensor_tensor(ab_blk, p_r, r_bc, ALU.mult)
                pending_fixup.clear()

            for iblk in range(IS // IBLK):
                pbuf = pbufp.tile([128, IBLK * JC * 16], F16, tag="pb")
                pbuf4 = pbuf[:].rearrange(
                    "p (i jc s) -> p i jc s", i=IBLK, jc=JC
                )
                musrc = smallp.tile([128, IBLK * JC], F16, tag="ms")
                ms3 = musrc[:].rearrange("p (i jc) -> p i jc", i=IBLK)
                sq = sqps.tile([128, IBLK * JC], F32, tag="sq")
                sq3 = sq[:].rearrange("p (i jc) -> p i jc", i=IBLK)
                for ii in range(0, IBLK, NI):
                    i = iblk * IBLK + ii
                    step = i // NI
                    etp = etpp.tile([128, NI * N], F16, tag="et")
                    nc.sync.dma_start(
                        etp[:].rearrange("p (u n) -> p u n", u=NI),
                        d_edge[b, i:i + NI, :, :].rearrange(
                            "u e n -> e u n"
                        ),
                    )
                    et4 = etp[:].rearrange(
                        "p (u jc j) -> p u jc j", u=NI, jc=JC
                    )
                    pps = ppps.tile([128, NI * JC * SW], F32, tag="pp")
                    pp4 = pps[:].rearrange(
                        "p (u jc s) -> p u jc s", u=NI, jc=JC
                    )
                    for u in range(NI):
                        for jc in range(JC):
                            nc.tensor.matmul(
                                pp4[:, u, jc, :], et4[:, u, jc, :],
                                wza[:], start=True, stop=True,
                            )
                    # sq-matmuls lagged 2 steps (square surely done by now
                    # even on the slower engines)
                    flush_sqmm(keep=1)
                    # square the tile in place (bias mms have consumed it);
                    # split across DVE / ACT / gpsimd
                    r = step % 16
                    if r in gp_steps:
                        nc.gpsimd.tensor_tensor(
                            etp[:], etp[:], etp[:], ALU.mult
                        )
                    elif r in act_steps:
                        nc.scalar.activation(
                            etp[:], etp[:], AF.Square, bias=0.0, scale=1.0,
                        )
                    else:
                        nc.vector.tensor_tensor(
                            etp[:], etp[:], etp[:], ALU.mult
                        )
                    pending_sqmm.append([
                        (sq3[:, ii + u, jc:jc + 1], et4[:, u, jc, :])
                        for u in range(NI) for jc in range(JC)
                    ])
                    # P copy psum->sbuf fp16 (bias cols only)
                    nc.scalar.activation(
                        pbuf4[:, ii:ii + NI, :, :], pp4[:, :, :, 0:16],
                        AF.Identity, bias=0.0, scale=1.0,
                    )
                    # sum_e x: tiny strided gather -> contiguous buf
                    nc.vector.tensor_copy(
                        ms3[:, ii:ii + NI, :], pp4[:, :, :, 16]
                    )
                    yield
                flush_sqmm()
                # ---- batched stats ---- var = sumsq/128 - (sum/128)^2
                mu = smallp.tile([128, IBLK * JC], F16, tag="mu")
                nc.vector.tensor_scalar_mul(mu[:], musrc[:], 1.0 / 128.0)
                ex2 = smallp.tile([128, IBLK * JC], F16, tag="e2")
                nc.vector.tensor_scalar_mul(ex2[:], sq[:], 1.0 / 128.0)
                musq = smallp.tile([128, IBLK * JC], F16, tag="m2")
                nc.vector.tensor_tensor(musq[:], mu[:], mu[:], ALU.mult)
                nc.vector.tensor_tensor(ex2[:], ex2[:], musq[:],
                                        ALU.subtract)
                lnv = lnvp.tile([128, IBLK * JC], F16, tag="lnv")
                nc.scalar.activation(
                    lnv[:], ex2[:], AF.Ln, bias=epsc[:, :], scale=1.0
                )
                # ---- fixup: abuf = P * rstd (mean folded into wza) ----
                ab_blk = abuf3[:, :, :, iblk * IBLK:(iblk + 1) * IBLK]
                p_r = pbuf4[:, :, :, :].rearrange("p i jc h -> p jc h i")
                pending_fixup.append((lnv, ab_blk, p_r))
                if iblk % 2 == 1:
                    flush_fixup()
            flush_fixup()

        # ============ attention generator ============
        def attn_gen(b, abuf3):
            oasm = oasmp.tile([128, DP], F16, tag="oa")
            # zero head-pad cols once (stale sbuf could hold NaN bits)
            oa4 = oasm[:].rearrange("p (h c) -> p h c", h=H)
            nc.vector.memset(oa4[:, :, HD:HDP], 0.0)
            for h in range(H):
                m = h // 2
                prow = (h % 2) * 64
                expsb = expsbp.tile([128, N], F16, tag="ex")
                ex3 = expsb[:].rearrange("p (jc i) -> p jc i", jc=JC)
                for half in range(2):
                    scp = mmps.tile([128, 512], F32, tag="sc")
                    sc3 = scp[:].rearrange("p (jc i) -> p jc i", jc=4)
                    for sj in range(4):
                        jc = half * 4 + sj
                        nc.tensor.matmul(
                            sc3[:, sj, :],
                            ktp3[:, b, m, jc * 128:(jc + 1) * 128][
                                prow:prow + 64, :
                            ],
                            qtp3[:, m, b, :][prow:prow + 64, :],
                            start=True, stop=True,
                        )
                    # add pair bias (DVE, psum rmw)
                    nc.vector.tensor_tensor(
                        sc3[:, :, :], sc3[:, :, :],
                        abuf3[:, half * 4:(half + 1) * 4, h, :],
                        ALU.add,
                    )
                    # exp -> sbuf fp16
                    nc.scalar.activation(
                        ex3[:, half * 4:(half + 1) * 4, :], sc3,
                        AF.Exp, bias=0.0, scale=1.0,
                    )
                # o+s = exp^T @ [v | ones] (accumulate over jc)
                opsum = ops.tile([128, VW], F32, tag="os")
                for jc in range(JC):
                    nc.tensor.matmul(
                        opsum[:],
                        ex3[:, jc, :],
                        vall4[:, b, jc, h, :],
                        start=(jc == 0), stop=(jc == JC - 1),
                    )
                sinv = smallp.tile([128, 1], F32, tag="si")
                nc.vector.reciprocal(sinv[:], opsum[:, HD:HD + 1])
                nc.scalar.activation(
                    oasm[:, h * HDP:h * HDP + HD],
                    opsum[:, 0:HD],
                    AF.Identity, bias=0.0, scale=sinv[:, :],
                )
                yield
            # go = g * o  (fp16, in place)
            nc.vector.tensor_tensor(oasm[:], oasm[:], gall2[:, b, :],
                                    ALU.mult)
            # transpose go -> goT chunks [d' rows, i cols]
            goT = goTp.tile([128, DP], F16, tag="goT")
            go3 = oasm[:].rearrange("p (c q) -> p c q", c=8)
            for cc in range(8):
                gops = ppps.tile([128, 128], F16, tag="pp")
                nc.tensor.transpose(gops[:], go3[:, cc, :], id16[:])
                nc.scalar.activation(
                    goT[:, cc * 128:(cc + 1) * 128], gops[:],
                    AF.Identity, bias=0.0, scale=1.0,
                )
            goT3 = goT[:].rearrange("p (c q) -> p c q", c=8)
            # final: out[i, :768] = goT.T @ wo
            outsb = outsbp.tile([128, D], F16, tag="ou")
            for nb, nsz in ((0, 512), (1, 256)):
                fps = mmps.tile([128, 512], F32, tag="sc")
                for cc in range(8):
                    nc.tensor.matmul(
                        fps[:, 0:nsz],
                        goT3[:, cc, :],
                        wosb2[:, cc, nb * 512:nb * 512 + nsz],
                        start=(cc == 0), stop=(cc == 7),
                    )
                nc.scalar.activation(
                    outsb[:, nb * 512:nb * 512 + nsz], fps[:, 0:nsz],
                    AF.Identity, bias=0.0, scale=1.0,
                )
            nc.sync.dma_start(d_out[b, :, :], outsb[:])
            yield

        # ============ interleaved drive ============
        abuf_t = {}
        ab3 = {}
        for b in range(B):
            abuf_b = abufp.tile([128, JC * H * IS], F16, tag="ab",
                                name=f"abuf{b}")
            abuf_t[b] = abuf_b
            ab3[b] = abuf_b[:].rearrange(
                "p (jc h i) -> p jc h i", jc=JC, h=H
            )

        def drive(primary, n_primary, secondary, n_secondary, every=1):
            pulled = 0
            step = 0
            for _ in primary:
                step += 1
                if step % every:
                    continue
                want = (step * n_secondary) // n_primary
                while pulled < want:
                    try:
                        next(secondary)
                        pulled += 1
                    except StopIteration:
                        pulled = n_secondary
                        break
            for _ in secondary:
                pass

        # [sweep(b0) || projections]
        drive(sweep_gen(0, ab3[0]), IS // NI, ph0_gen(), 76)
        # [sweep(b1) || attn(b0)]; attn chunks pulled at iblk boundaries
        # so attn Exp ops sit adjacent to the batched Ln/Exp stats ops
        # (activation-table locality). DVE serves the attn bias-adds
        # here, so more squares go to gpsimd/ACT.
        drive(sweep_gen(1, ab3[1], gp_steps=(0, 4, 8, 12),
                        act_steps=(2, 6, 10, 14)),
              IS // NI, attn_gen(0, ab3[0]), 17,
              every=IBLK // NI)
        # attn(b1)
        for _ in attn_gen(1, ab3[1]):
            pass

    nc.compile()
    return nc


def _prep_host(inputs):
    """Build per-core input maps (host-side layout marshalling only)."""
    node = np.asarray(inputs["node_embed"], np.float32)
    edge = np.asarray(inputs["edge_embed"], np.float32)
    mask = np.asarray(inputs["node_mask"])
    k_in = np.asarray(inputs["k_in"], np.float32)
    Wq = np.asarray(inputs["Wq"], np.float32)
    bq = np.asarray(inputs["bq"], np.float32)
    Wk = np.asarray(inputs["Wk"], np.float32)
    Wv = np.asarray(inputs["Wv"], np.float32)
    Wg = np.asarray(inputs["Wg"], np.float32)
    ln_g = np.asarray(inputs["ln_g"], np.float32)
    ln_b = np.asarray(inputs["ln_b"], np.float32)
    Wz = np.asarray(inputs["Wz"], np.float32)
    Wo = np.asarray(inputs["Wo"], np.float32)

    assert np.all(np.asarray(mask) == 1), "mask path not implemented"
    assert np.all(ln_b == 0.0), "nonzero ln_b not implemented"

    scale = 1.0 / np.sqrt(HD)

    def padhead_rows(W):  # (768,768) -> (1024,768): out' rows padded
        Wp = np.zeros((DP, D), np.float32)
        for h in range(H):
            Wp[h * HDP:h * HDP + HD] = W[h * HD:(h + 1) * HD]
        return Wp

    wqT = (padhead_rows(Wq) * scale).T.astype(np.float16).copy()
    wkT = padhead_rows(Wk).T.astype(np.float16).copy()
    wvT = padhead_rows(Wv).T.astype(np.float16).copy()
    wgT = (-Wg).T.astype(np.float16).copy()  # negated; (768,768)->pad cols
    wgTp = np.zeros((D, DP), np.float16)
    for h in range(H):
        wgTp[:, h * HDP:h * HDP + HD] = wgT[:, h * HD:(h + 1) * HD]
    woTp = np.zeros((DP, D), np.float32)
    WoT = Wo.T  # (d_in=768, d_out=768); d_in is the g*o dim
    for h in range(H):
        woTp[h * HDP:h * HDP + HD] = WoT[h * HD:(h + 1) * HD]
    woTp = woTp.astype(np.float16)

    bqp = np.zeros((DP,), np.float32)
    for h in range(H):
        bqp[h * HDP:h * HDP + HD] = bq[h * HD:(h + 1) * HD] * scale
    bqp = bqp.reshape(DP // 128, 128)

    wza = np.zeros((E, SW), np.float32)
    gwz = ln_g[:, None] * Wz
    # column-demeaned: folds the LN mean subtraction into the weights
    wza[:, :16] = gwz - gwz.mean(axis=0, keepdims=True)
    wza[:, 16] = 1.0
    wza16 = wza.astype(np.float16)

    xt = node.transpose(0, 2, 1).astype(np.float16).copy()     # (B, D, N)
    kinT = k_in.transpose(0, 2, 1).astype(np.float16).copy()   # (B, D, N)
    edge16 = edge.astype(np.float16)

    id16 = np.eye(128, dtype=np.float16)

    in_maps = []
    for c in range(NC):
        i0 = c * IS
        in_maps.append({
            # (B, IS, E, N): [e, j] tiles
            "e": np.ascontiguousarray(
                edge16[:, i0:i0 + IS].transpose(0, 1, 3, 2)
            ),
            "xt": np.ascontiguousarray(xt[:, :, i0:i0 + IS]),
            "kin": kinT,
            "wq": wqT, "wk": wkT, "wv": wvT, "wg": wgTp, "wo": woTp,
            "bq": bqp, "wza": wza16,
            "id16": id16,
        })
    return in_maps


def kernel(**inputs):
    global _BUILT, LAST_RESULTS
    if _BUILT is None:
        _BUILT = _build_program()
    nc = _BUILT
    in_maps = _prep_host(inputs)
    res = run_bass_kernel_spmd(
        nc, in_maps, core_ids=list(range(NC)),
        trace=bool(int(os.environ.get("KERNEL_TRACE", "0"))),
    )
    LAST_RESULTS = res
    out = np.empty((B, N, D), np.float32)
    for c in range(NC):
        out[:, c * IS:(c + 1) * IS] = np.asarray(
            res.results[c]["o"], np.float32
        )
    return out


if __name__ == "__main__":
    sys.path.insert(0, os.path.dirname(os.path.abspath(__file__)))
    import reference
    inputs = {k: np.asarray(v) for k, v in reference.setup_inputs().items()}
    got = kernel(**inputs)
    want = np.asarray(reference.reference(**reference.setup_inputs()))
    err = np.abs(got - want)
    rel = np.abs(got - want) / (np.abs(want).mean() + 1e-9)
    print("max abs err:", err.max(), "rel:", rel.max())


# revision 52
# speedup vs baseline: 1.0512x; 1.0512x over previous
"""AttentionPairBias Trainium2 Bass kernel.

Problem: nn_AttentionPairBias_49486613184627
  B=2, N=1024, D=768, E=128, H=16, HD=48.

Sharding: query-row (i) sharding across 8 cores. Core c handles rows
i in [c*128, (c+1)*128) for both batches. Each core reads its edge_embed
shard (67MB in fp16), full k_in (dup k/v projection), and produces its
(2,128,768) slice of the output.

Device-side layout strategy ("etp", interleaved emission):
  - edge is pre-transposed on host to (B, IS, E, N): tiles load as
    [e(part), j(free)] directly -> no PE transposes, no psum->sbuf
    copies for the bias path.
  - pair bias per (b,i,jc): one matmul lhsT=edge_jc rhs=[wza' | ones]
    where wza' = ln_g*Wz column-demeaned on host -- this folds the
    LayerNorm mean subtraction into the weights (sum_e (x-mu)w =
    sum_e x(w-mean(w))). P[j,0:16]=mean-centered bias, P[j,16]=sum_e x
    (for the variance only). The edge tile is squared (split across
    DVE/ACT/GpSimd); a matmul lhsT=esq_jc rhs=ones accumulates
    sum_e x^2 into a per-16-i-block stats psum tile (contiguous). The
    LayerNorm scale is a single batched fixup bias = P*rstd (GpSimd).
  - scores tiles are [j(part), i(free)] per (b,h,jc); bias added on DVE,
    exp on ACT -> fp16 sbuf.
  - o = exp^T @ [v | ones]: ones column fused into v (49-wide heads) so
    col 48 of the o-psum is the softmax denominator; 1/s is a
    per-partition ACT scale.
  - emission is interleaved so engines overlap across phases:
    [proj-phase || bias-sweep(b0)], then [attn(b0) || bias-sweep(b1)],
    then attn(b1); abuf is double-buffered to decouple the batches.
  - ACT uses only Identity/Ln/Exp/Square -> one table set, no reloads.
    (sigmoid computed as 1/(1+exp(-z)) with DVE reciprocal)
"""

import os
import sys

import numpy as np

for _p in ("/opt/trn_rl_repo",):
    if _p not in sys.path:
        sys.path.insert(0, _p)

import concourse.bacc as bacc
import concourse.bass as bass
import concourse.mybir as mybir
import concourse.tile as tile
from concourse.bass_utils import run_bass_kernel_spmd

F16 = mybir.dt.float16
F32 = mybir.dt.float32
AF = mybir.ActivationFunctionType
ALU = mybir.AluOpType

B, N, D, E, H = 2, 1024, 768, 128, 16
HD = 48
HDP = 64              # padded head dim (projection layout)
VW = HD + 1           # v width per head incl. ones column
DP = H * HDP          # 1024 padded model dim
NC = 8                # cores
IS = N // NC          # 128 i-rows per core per batch
JC = N // 128         # 8 j-chunks
MC = D // 128         # 6 contraction chunks of 128 over D
IBLK = 16             # i-batch for stats/fixup
SW = 17               # bias matmul width: 16 bias cols + sum_e x
EPS = 1e-5

_BUILT = None         # cached program
LAST_RESULTS = None   # BassKernelResults of last run (for test.py)


def _build_program():
    nc = bacc.Bacc(
        "TRN2",
        target_bir_lowering=False,
        debug=False,
        enable_asserts=False,
        num_devices=NC,
    )

    # ---------------- DRAM I/O ----------------
    d_edge = nc.dram_tensor("e", (B, IS, E, N), F16, kind="ExternalInput").ap()
    d_xt = nc.dram_tensor("xt", (B, D, IS), F16, kind="ExternalInput").ap()
    d_kin = nc.dram_tensor("kin", (B, D, N), F16, kind="ExternalInput").ap()
    d_wq = nc.dram_tensor("wq", (D, DP), F16, kind="ExternalInput").ap()
    d_wk = nc.dram_tensor("wk", (D, DP), F16, kind="ExternalInput").ap()
    d_wv = nc.dram_tensor("wv", (D, DP), F16, kind="ExternalInput").ap()
    d_wg = nc.dram_tensor("wg", (D, DP), F16, kind="ExternalInput").ap()
    d_wo = nc.dram_tensor("wo", (DP, D), F16, kind="ExternalInput").ap()
    d_bq = nc.dram_tensor("bq", (HDP * H // 128, 128), F32, kind="ExternalInput").ap()
    d_wza = nc.dram_tensor("wza", (E, SW), F16, kind="ExternalInput").ap()
    d_id16 = nc.dram_tensor("id16", (128, 128), F16, kind="ExternalInput").ap()
    d_out = nc.dram_tensor("o", (B, IS, D), F16, kind="ExternalOutput").ap()

    from contextlib import ExitStack

    with tile.TileContext(nc) as tc, ExitStack() as es:
        def pool(**kw):
            return es.enter_context(tc.tile_pool(**kw))

        # ---- SBUF pools ----
        constp = pool(name="const", bufs=1)
        ktpp = pool(name="ktp", bufs=1)
        vallp = pool(name="vall", bufs=1)
        qtpp = pool(name="qtp", bufs=1)
        gallp = pool(name="gall", bufs=1)
        wosbp = pool(name="wo_sb", bufs=1)
        wchp = pool(name="wchunk", bufs=6)
        kinchp = pool(name="kinchunk", bufs=6)
        gwork = pool(name="gwork", bufs=1)
        abufp = pool(name="abuf", bufs=2)
        etpp = pool(name="etp", bufs=3)
        pbufp = pool(name="pbuf", bufs=3)
        smallp = pool(name="small", bufs=2)
        lnvp = pool(name="lnv", bufs=3)
        expsbp = pool(name="expsb", bufs=2)
        oasmp = pool(name="oasm", bufs=1)
        goTp = pool(name="goT", bufs=1)
        outsbp = pool(name="outsb", bufs=1)
        # ---- PSUM pools (8 banks) ----
        ppps = pool(name="pp_ps", bufs=2, space="PSUM")   # P: f32 136
        sqps = pool(name="sq_ps", bufs=2, space="PSUM")   # sumsq f32 128
        mmps = pool(name="mm_ps", bufs=2, space="PSUM")   # proj/scores f32 512
        ops = pool(name="o_ps", bufs=2, space="PSUM")     # o+s: f32 49

        # ============ constants ============
        id16 = constp.tile([128, 128], F16)
        nc.sync.dma_start(id16[:], d_id16[:, :])
        wza = constp.tile([E, SW], F16)
        nc.sync.dma_start(wza[:], d_wza[:, :])
        bqp = constp.tile([128, DP // 128], F32)
        nc.sync.dma_start(bqp[:], d_bq.rearrange("m p -> p m"))
        ones16 = constp.tile([128, 1], F16)
        nc.vector.memset(ones16[:], 1.0)
        epsc = constp.tile([128, 1], F32)
        nc.vector.memset(epsc[:], EPS)

        # persistent activation buffers
        # ktp: [b][m] 128 x 1024 (d' rows, j cols), fp16
        ktp = ktpp.tile([128, B * 8 * 1024], F16)
        ktp3 = ktp[:].rearrange("p (b m j) -> p b m j", b=B, m=8)
        # v: [b][jt][h] 128 x 49 (j rows, [v|ones] cols), fp16
        vall = vallp.tile([128, B * 8 * H * VW], F16)
        vall4 = vall[:].rearrange(
            "p (b jt h c) -> p b jt h c", b=B, jt=8, h=H
        )
        nc.vector.memset(vall4[:, :, :, :, HD:HD + 1], 1.0)
        # qtp: [m] 128 x (b,i), fp16
        qtp = qtpp.tile([128, 8 * B * IS], F16)
        qtp3 = qtp[:].rearrange("p (m b i) -> p m b i", m=8, b=B)
        # g: [b] 128(i) x 1024(d'), fp16
        gall = gallp.tile([128, B * DP], F16)
        gall2 = gall[:].rearrange("p (b d) -> p b d", b=B)
        # wo chunks: [cc] 128 x 768 fp16
        wosb = wosbp.tile([128, 8 * D], F16)
        wosb2 = wosb[:].rearrange("p (c d) -> p c d", c=8)
        nc.sync.dma_start(wosb2, d_wo.rearrange("(c p) d -> p c d", p=128))
        # xt tiles: [c] 128(d-row) x (b,i)
        xts = constp.tile([128, MC * B * IS], F16)
        xts3 = xts[:].rearrange("p (c b i) -> p c b i", c=MC, b=B)
        for b in range(B):
            for c in range(MC):
                nc.sync.dma_start(
                    xts3[:, c, b, :], d_xt[b, c * 128:(c + 1) * 128, :]
                )

        # ============ phase 0 generator: projections ============
        def load_chunks(dram, tag, n=MC, width=DP):
            ts = []
            for c in range(n):
                t = wchp.tile([128, width], F16, tag=tag)
                nc.sync.dma_start(t[:], dram[c * 128:(c + 1) * 128, :])
                ts.append(t)
            return ts

        def ph0_gen():
            # q projection (both b at once; xts free dim is (b,i))
            wq_sb = load_chunks(d_wq, "w")
            for m in range(8):
                qps = mmps.tile([128, B * IS], F32, tag="sc")
                for c in range(MC):
                    nc.tensor.matmul(
                        qps[:],
                        wq_sb[c][:, m * 128:(m + 1) * 128],
                        xts3[:, c, :, :],
                        start=(c == 0), stop=(c == MC - 1),
                    )
                nc.scalar.activation(
                    qtp3[:, m, :, :], qps[:],
                    AF.Identity, bias=bqp[:, m:m + 1], scale=1.0,
                )
                yield
            # g = 1/(1+exp(-z)); wg is pre-negated on host -> psum = -z
            wg_sb = load_chunks(d_wg, "w")
            for b in range(B):
                for nb in range(2):
                    gps = mmps.tile([128, 512], F32, tag="sc")
                    for c in range(MC):
                        nc.tensor.matmul(
                            gps[:],
                            xts3[:, c, b, :],
                            wg_sb[c][:, nb * 512:(nb + 1) * 512],
                            start=(c == 0), stop=(c == MC - 1),
                        )
                    gtmp = gwork.tile([128, 512], F32, tag="gtmp")
                    nc.scalar.activation(
                        gtmp[:], gps[:], AF.Exp, bias=0.0, scale=1.0,
                    )
                    nc.vector.tensor_scalar_add(gtmp[:], gtmp[:], 1.0)
                    with nc.allow_low_precision(
                        reason="sigmoid gate in (0,1); fp16 out is ample"
                    ):
                        nc.vector.reciprocal(
                            gall2[:, b, nb * 512:(nb + 1) * 512], gtmp[:]
                        )
                    yield
            # per-b: k^T and v
            for b in range(B):
                kin_sb = []
                for c in range(MC):
                    t = kinchp.tile([128, N], F16, tag="kin")
                    nc.sync.dma_start(
                        t[:], d_kin[b, c * 128:(c + 1) * 128, :]
                    )
                    kin_sb.append(t)
                wk_sb = load_chunks(d_wk, "w")
                for m in range(8):
                    for nb in range(2):
                        kps = mmps.tile([128, 512], F32, tag="sc")
                        for c in range(MC):
                            nc.tensor.matmul(
                                kps[:],
                                wk_sb[c][:, m * 128:(m + 1) * 128],
                                kin_sb[c][:, nb * 512:(nb + 1) * 512],
                                start=(c == 0), stop=(c == MC - 1),
                            )
                        nc.scalar.activation(
                            ktp3[:, b, m, nb * 512:(nb + 1) * 512], kps[:],
                            AF.Identity, bias=0.0, scale=1.0,
                        )
                        yield
                wv_sb = load_chunks(d_wv, "w")
                for jt in range(8):
                    for nb in range(2):
                        vps = mmps.tile([128, 512], F32, tag="sc")
                        for c in range(MC):
                            nc.tensor.matmul(
                                vps[:],
                                kin_sb[c][:, jt * 128:(jt + 1) * 128],
                                wv_sb[c][:, nb * 512:(nb + 1) * 512],
                                start=(c == 0), stop=(c == MC - 1),
                            )
                        nc.scalar.activation(
                            vall4[:, b, jt, nb * 8:(nb + 1) * 8, 0:HD],
                            vps[:].rearrange(
                                "p (h c) -> p h c", h=8
                            )[:, :, 0:HD],
                            AF.Identity, bias=0.0, scale=1.0,
                        )
                        yield

        # ============ bias sweep generator ============
        NI = 2                      # i-rows per sweep step
        def sweep_gen(b, abuf3, gp_steps=(0, 5, 10), act_steps=(3, 8, 13)):
            # deferred emission: sq-matmuls one step late; rstd Exp +
            # fixup batched per 2 iblks (batches the Ln ops apart from
            # the Exp ops -> far fewer activation-table reloads, and no
            # engine queue head-of-line-blocks on a fresh dependency)
            pending_sqmm = []   # list of per-step lists; flushed lag-2
            pending_fixup = []

            def flush_sqmm(keep=0):
                while len(pending_sqmm) > keep:
                    for out_ap, lhsT in pending_sqmm.pop(0):
                        nc.tensor.matmul(out_ap, lhsT, ones16[:],
                                         start=True, stop=True)

            def flush_fixup():
                for lnv, ab_blk, p_r in pending_fixup:
                    rstd = lnvp.tile([128, IBLK * JC], F16, tag="rs")
                    nc.scalar.activation(
                        rstd[:], lnv[:], AF.Exp, bias=0.0, scale=-0.5
                    )
                    rstd3 = rstd[:].rearrange(
                        "p (i jc) -> p i jc", i=IBLK
                    )
                    r_bc = rstd3.rearrange(
                        "p i jc -> p jc () i"
                    ).broadcast_to((128, JC, 16, IBLK))
                    nc.vector.tensor_tensor(ab_blk, p_r, r_bc, ALU.mult)
                pending_fixup.clear()

            for iblk in range(IS // IBLK):
                pbuf = pbufp.tile([128, IBLK * JC * 16], F16, tag="pb")
                pbuf4 = pbuf[:].rearrange(
                    "p (i jc s) -> p i jc s", i=IBLK, jc=JC
                )
                musrc = smallp.tile([128, IBLK * JC], F16, tag="ms")
                ms3 = musrc[:].rearrange("p (i jc) -> p i jc", i=IBLK)
                sq = sqps.tile([128, IBLK * JC], F32, tag="sq")
                sq3 = sq[:].rearrange("p (i jc) -> p i jc", i=IBLK)
                for ii in range(0, IBLK, NI):
                    i = iblk * IBLK + ii
                    step = i // NI
                    etp = etpp.tile([128, NI * N], F16, tag="et")
                    nc.sync.dma_start(
                        etp[:].rearrange("p (u n) -> p u n", u=NI),
                        d_edge[b, i:i + NI, :, :].rearrange(
                            "u e n -> e u n"
                        ),
                    )
                    et4 = etp[:].rearrange(
                        "p (u jc j) -> p u jc j", u=NI, jc=JC
                    )
                    pps = ppps.tile([128, NI * JC * SW], F32, tag="pp")
                    pp4 = pps[:].rearrange(
                        "p (u jc s) -> p u jc s", u=NI, jc=JC
                    )
                    for u in range(NI):
                        for jc in range(JC):
                            nc.tensor.matmul(
                                pp4[:, u, jc, :], et4[:, u, jc, :],
                                wza[:], start=True, stop=True,
                            )
                    # sq-matmuls lagged 2 steps (square surely done by now
                    # even on the slower engines)
                    flush_sqmm(keep=1)
                    # square the tile in place (bias mms have consumed it);
                    # split across DVE / ACT / gpsimd
                    r = step % 16
                    if r in gp_steps:
                        nc.gpsimd.tensor_tensor(
                            etp[:], etp[:], etp[:], ALU.mult
                        )
                    elif r in act_steps:
                        nc.scalar.activation(
                            etp[:], etp[:], AF.Square, bias=0.0, scale=1.0,
                        )
                    else:
                        nc.vector.tensor_tensor(
                            etp[:], etp[:], etp[:], ALU.mult
                        )
                    pending_sqmm.append([
                        (sq3[:, ii + u, jc:jc + 1], et4[:, u, jc, :])
                        for u in range(NI) for jc in range(JC)
                    ])
                    # P copy psum->sbuf fp16 (bias cols only)
                    nc.scalar.activation(
                        pbuf4[:, ii:ii + NI, :, :], pp4[:, :, :, 0:16],
                        AF.Identity, bias=0.0, scale=1.0,
                    )
                    # sum_e x: tiny strided gather -> contiguous buf
                    nc.vector.tensor_copy(
                        ms3[:, ii:ii + NI, :], pp4[:, :, :, 16]
                    )
                    yield
                flush_sqmm()
                # ---- batched stats ---- var = sumsq/128 - (sum/128)^2
                mu = smallp.tile([128, IBLK * JC], F16, tag="mu")
                nc.vector.tensor_scalar_mul(mu[:], musrc[:], 1.0 / 128.0)
                ex2 = smallp.tile([128, IBLK * JC], F16, tag="e2")
                nc.vector.tensor_scalar_mul(ex2[:], sq[:], 1.0 / 128.0)
                musq = smallp.tile([128, IBLK * JC], F16, tag="m2")
                nc.vector.tensor_tensor(musq[:], mu[:], mu[:], ALU.mult)
                nc.vector.tensor_tensor(ex2[:], ex2[:], musq[:],
                                        ALU.subtract)
                lnv = lnvp.tile([128, IBLK * JC], F16, tag="lnv")
                nc.scalar.activation(
                    lnv[:], ex2[:], AF.Ln, bias=epsc[:, :], scale=1.0
                )
                # ---- fixup: abuf = P * rstd (mean folded into wza) ----
                ab_blk = abuf3[:, :, :, iblk * IBLK:(iblk + 1) * IBLK]
                p_r = pbuf4[:, :, :, :].rearrange("p i jc h -> p jc h i")
                pending_fixup.append((lnv, ab_blk, p_r))
                if iblk % 2 == 1:
                    flush_fixup()
            flush_fixup()

        # ============ attention generator ============
        def attn_gen(b, abuf3):
            oasm = oasmp.tile([128, DP], F16, tag="oa")
            # zero head-pad cols once (stale sbuf could hold NaN bits)
            oa4 = oasm[:].rearrange("p (h c) -> p h c", h=H)
            nc.vector.memset(oa4[:, :, HD:HDP], 0.0)
            for h in range(H):
                m = h // 2
                prow = (h % 2) * 64
                expsb = expsbp.tile([128, N], F16, tag="ex")
                ex3 = expsb[:].rearrange("p (jc i) -> p jc i", jc=JC)
                for half in range(2):
                    scp = mmps.tile([128, 512], F32, tag="sc")
                    sc3 = scp[:].rearrange("p (jc i) -> p jc i", jc=4)
                    for sj in range(4):
                        jc = half * 4 + sj
                        nc.tensor.matmul(
                            sc3[:, sj, :],
                            ktp3[:, b, m, jc * 128:(jc + 1) * 128][
                                prow:prow + 64, :
                            ],
                            qtp3[:, m, b, :][prow:prow + 64, :],
                            start=True, stop=True,
                        )
                    # add pair bias (DVE, psum rmw)
                    nc.vector.tensor_tensor(
                        sc3[:, :, :], sc3[:, :, :],
                        abuf3[:, half * 4:(half + 1) * 4, h, :],
                        ALU.add,
                    )
                    # exp -> sbuf fp16
                    nc.scalar.activation(
                        ex3[:, half * 4:(half + 1) * 4, :], sc3,
                        AF.Exp, bias=0.0, scale=1.0,
                    )
                # o+s = exp^T @ [v | ones] (accumulate over jc)
                opsum = ops.tile([128, VW], F32, tag="os")
                for jc in range(JC):
                    nc.tensor.matmul(
                        opsum[:],
                        ex3[:, jc, :],
                        vall4[:, b, jc, h, :],
                        start=(jc == 0), stop=(jc == JC - 1),
                    )
                sinv = smallp.tile([128, 1], F32, tag="si")
                nc.vector.reciprocal(sinv[:], opsum[:, HD:HD + 1])
                nc.scalar.activation(
                    oasm[:, h * HDP:h * HDP + HD],
                    opsum[:, 0:HD],
                    AF.Identity, bias=0.0, scale=sinv[:, :],
                )
                yield
            # go = g * o  (fp16, in place)
            nc.vector.tensor_tensor(oasm[:], oasm[:], gall2[:, b, :],
                                    ALU.mult)
            # transpose go -> goT chunks [d' rows, i cols]
            goT = goTp.tile([128, DP], F16, tag="goT")
            go3 = oasm[:].rearrange("p (c q) -> p c q", c=8)
            for cc in range(8):
                gops = ppps.tile([128, 128], F16, tag="pp")
                nc.tensor.transpose(gops[:], go3[:, cc, :], id16[:])
                nc.scalar.activation(
                    goT[:, cc * 128:(cc + 1) * 128], gops[:],
                    AF.Identity, bias=0.0, scale=1.0,
                )
            goT3 = goT[:].rearrange("p (c q) -> p c q", c=8)
            # final: out[i, :768] = goT.T @ wo
            outsb = outsbp.tile([128, D], F16, tag="ou")
            for nb, nsz in ((0, 512), (1, 256)):
                fps = mmps.tile([128, 512], F32, tag="sc")
                for cc in range(8):
                    nc.tensor.matmul(
                        fps[:, 0:nsz],
                        goT3[:, cc, :],
                        wosb2[:, cc, nb * 512:nb * 512 + nsz],
                        start=(cc == 0), stop=(cc == 7),
                    )
                nc.scalar.activation(
                    outsb[:, nb * 512:nb * 512 + nsz], fps[:, 0:nsz],
                    AF.Identity, bias=0.0, scale=1.0,
                )
            nc.sync.dma_start(d_out[b, :, :], outsb[:])
            yield

        # ============ interleaved drive ============
        abuf_t = {}
        ab3 = {}
        for b in range(B):
            abuf_b = abufp.tile([128, JC * H * IS], F16, tag="ab",
                                name=f"abuf{b}")
            abuf_t[b] = abuf_b
            ab3[b] = abuf_b[:].rearrange(
                "p (jc h i) -> p jc h i", jc=JC, h=H
            )

        def drive(primary, n_primary, secondary, n_secondary, every=1):
            pulled = 0
            step = 0
            for _ in primary:
                step += 1
                if step % every:
                    continue
                want = (step * n_secondary) // n_primary
                while pulled < want:
                    try:
                        next(secondary)
                        pulled += 1
                    except StopIteration:
                        pulled = n_secondary
                        break
            for _ in secondary:
                pass

        # [sweep(b0) || projections]
        drive(sweep_gen(0, ab3[0]), IS // NI, ph0_gen(), 76)
        # [sweep(b1) || attn(b0)]; attn chunks pulled at iblk boundaries
        # so attn Exp ops sit adjacent to the batched Ln/Exp stats ops
        # (activation-table locality)
        drive(sweep_gen(1, ab3[1]), IS // NI, attn_gen(0, ab3[0]), 17,
              every=IBLK // NI)
        # attn(b1)
        for _ in attn_gen(1, ab3[1]):
            pass

    nc.compile()
    return nc


def _prep_host(inputs):
    """Build per-core input maps (host-side layout marshalling only)."""
    node = np.asarray(inputs["node_embed"], np.float32)
    edge = np.asarray(inputs["edge_embed"], np.float32)
    mask = np.asarray(inputs["node_mask"])
    k_in = np.asarray(inputs["k_in"], np.float32)
    Wq = np.asarray(inputs["Wq"], np.float32)
    bq = np.asarray(inputs["bq"], np.float32)
    Wk = np.asarray(inputs["Wk"], np.float32)
    Wv = np.asarray(inputs["Wv"], np.float32)
    Wg = np.asarray(inputs["Wg"], np.float32)
    ln_g = np.asarray(inputs["ln_g"], np.float32)
    ln_b = np.asarray(inputs["ln_b"], np.float32)
    Wz = np.asarray(inputs["Wz"], np.float32)
    Wo = np.asarray(inputs["Wo"], np.float32)

    assert np.all(np.asarray(mask) == 1), "mask path not implemented"
    assert np.all(ln_b == 0.0), "nonzero ln_b not implemented"

    scale = 1.0 / np.sqrt(HD)

    def padhead_rows(W):  # (768,768) -> (1024,768): out' rows padded
        Wp = np.zeros((DP, D), np.float32)
        for h in range(H):
            Wp[h * HDP:h * HDP + HD] = W[h * HD:(h + 1) * HD]
        return Wp

    wqT = (padhead_rows(Wq) * scale).T.astype(np.float16).copy()
    wkT = padhead_rows(Wk).T.astype(np.float16).copy()
    wvT = padhead_rows(Wv).T.astype(np.float16).copy()
    wgT = (-Wg).T.astype(np.float16).copy()  # negated; (768,768)->pad cols
    wgTp = np.zeros((D, DP), np.float16)
    for h in range(H):
        wgTp[:, h * HDP:h * HDP + HD] = wgT[:, h * HD:(h + 1) * HD]
    woTp = np.zeros((DP, D), np.float32)
    WoT = Wo.T  # (d_in=768, d_out=768); d_in is the g*o dim
    for h in range(H):
        woTp[h * HDP:h * HDP + HD] = WoT[h * HD:(h + 1) * HD]
    woTp = woTp.astype(np.float16)

    bqp = np.zeros((DP,), np.float32)
    for h in range(H):
        bqp[h * HDP:h * HDP + HD] = bq[h * HD:(h + 1) * HD] * scale
    bqp = bqp.reshape(DP // 128, 128)

    wza = np.zeros((E, SW), np.float32)
    gwz = ln_g[:, None] * Wz
    # column-demeaned: folds the LN mean subtraction into the weights
    wza[:, :16] = gwz - gwz.mean(axis=0, keepdims=True)
    wza[:, 16] = 1.0
    wza16 = wza.astype(np.float16)

    xt = node.transpose(0, 2, 1).astype(np.float16).copy()     # (B, D, N)
    kinT = k_in.transpose(0, 2, 1).astype(np.float16).copy()   # (B, D, N)
    edge16 = edge.astype(np.float16)

    id16 = np.eye(128, dtype=np.float16)

    in_maps = []
    for c in range(NC):
        i0 = c * IS
        in_maps.append({
            # (B, IS, E, N): [e, j] tiles
            "e": np.ascontiguousarray(
                edge16[:, i0:i0 + IS].transpose(0, 1, 3, 2)
            ),
            "xt": np.ascontiguousarray(xt[:, :, i0:i0 + IS]),
            "kin": kinT,
            "wq": wqT, "wk": wkT, "wv": wvT, "wg": wgTp, "wo": woTp,
            "bq": bqp, "wza": wza16,
            "id16": id16,
        })
    return in_maps


def kernel(**inputs):
    global _BUILT, LAST_RESULTS
    if _BUILT is None:
        _BUILT = _build_program()
    nc = _BUILT
    in_maps = _prep_host(inputs)
    res = run_bass_kernel_spmd(
        nc, in_maps, core_ids=list(range(NC)),
        trace=bool(int(os.environ.get("KERNEL_TRACE", "0"))),
    )
    LAST_RESULTS = res
    out = np.empty((B, N, D), np.float32)
    for c in range(NC):
        out[:, c * IS:(c + 1) * IS] = np.asarray(
            res.results[c]["o"], np.float32
        )
    return out


if __name__ == "__main__":
    sys.path.insert(0, os.path.dirname(os.path.abspath(__file__)))
    import reference
    inputs = {k: np.asarray(v) for k, v in reference.setup_inputs().items()}
    got = kernel(**inputs)
    want = np.asarray(reference.reference(**reference.setup_inputs()))
    err = np.abs(got - want)
    rel = np.abs(got - want) / (np.abs(want).mean() + 1e-9)
    print("max abs err:", err.max(), "rel:", rel.max())
